# revision 34
# baseline (speedup 1.0000x reference)
"""MoE SwiGLU experts (T=2048, H=2048, I=5632, E=8, top-2) on 8 trn2 cores.

Strategy: expert-parallel routed compute. The reference computes all 8
experts densely for every token, but the output only needs each token's
top-2 experts, so we gather tokens per expert on the host (merging the
case where both top-k slots pick the same expert), run one expert per
NeuronCore on its ~T*K/E gathered tokens, and scatter-combine with the
router weights on the host.  4x less device FLOPs than dense.

Per core (expert e), with C = padded token capacity:
  phase 1: hT[i, c] = silu(w1[e].T @ xgT) * (w3[e].T @ xgT)   [I, C]
           - accumulate over 16 H-chunks of 128 in PSUM, f32r matmuls
  phase 2: y[c, h]  = hT.T @ w2[e]                            [C, H]
           - accumulate over 44 I-chunks of 128 in PSUM
All matmuls use float32r (full PE rate at moving-dim >= 256, ~1.4e-4
max rel err vs fp32).  Weights are host-retiled so every DMA is one
contiguous [128, wtile] block (2KB/partition lines when C <= 512).
"""

import numpy as np

import concourse.bacc as bacc
import concourse.mybir as mybir
import concourse.tile as tile
from concourse.bass_utils import run_bass_kernel_spmd

E = 8
H = 2048
I = 5632
HK = H // 128   # 16 contraction chunks for phase 1
IK = I // 128   # 44 contraction chunks for phase 2
HG = H // 512   # 4 output column groups (w2)
C_CAP = 640     # max tokens per expert per round (SBUF budget)

F32 = mybir.dt.float32
F32R = mybir.dt.float32r
SILU = mybir.ActivationFunctionType.Silu

_prog_cache: dict[int, object] = {}


def _chunk_list(c):
    """Split c (multiple of 128) into moving-dim chunks of at most 512
    (PSUM bank), preferring >=256 so f32r matmuls run at full rate."""
    out, off, r = [], 0, c
    while r > 0:
        if r <= 512:
            t = r
        elif r < 768:
            t = r - 256
        else:
            t = 512
        out.append((off, t))
        off += t
        r -= t
    return out


def _wtile(c):
    """Phase-1 weight tile width: 2 weights x (wtile/128) ic-tiles x
    n_chunks accumulation groups must fit in 8 PSUM banks."""
    return 512 if len(_chunk_list(c)) == 1 else 256


def _build(c):
    nc = bacc.Bacc("TRN2", target_bir_lowering=False, debug=False, num_devices=E)
    wt_w = _wtile(c)
    icpt = wt_w // 128          # ic-tiles per phase-1 weight tile
    n_icg = I // wt_w           # phase-1 weight groups
    xgT = nc.dram_tensor("xgT", [HK, 128, c], F32R, kind="ExternalInput")
    w1 = nc.dram_tensor(
        "w1", [n_icg, HK // 2, 128, 2, wt_w], F32R, kind="ExternalInput"
    )
    w3 = nc.dram_tensor(
        "w3", [n_icg, HK // 2, 128, 2, wt_w], F32R, kind="ExternalInput"
    )
    w2 = nc.dram_tensor("w2", [HG, IK, 128, 512], F32R, kind="ExternalInput")
    y = nc.dram_tensor("y", [c, H], F32, kind="ExternalOutput")
    scratch = nc.dram_tensor("scratch", [128, 512], F32, kind="ExternalOutput")

    ch = _chunk_list(c)
    tt_n = c // 128
    wbufs = 6

    with tile.TileContext(nc) as tc:
        with (
            tc.tile_pool(name="xg", bufs=1) as xpool,
            tc.tile_pool(name="h", bufs=1) as hpool,
            tc.tile_pool(name="w", bufs=wbufs) as wpool,
            tc.tile_pool(name="w2p", bufs=8) as w2pool,
            tc.tile_pool(name="ps", bufs=8, space="PSUM") as pspool,
            tc.tile_pool(name="o", bufs=4) as opool,
        ):
            # PE warmup: matmuls on a zeroed tile keep the PE busy (and the
            # HAM clock un-throttled) while the first input DMAs land.
            wu0 = xpool.tile([128, 512], F32, tag="wu0", name="wu0")
            nc.vector.memset(wu0[:], 0.0)
            wu = xpool.tile([128, 512], F32R, tag="wu", name="wu")
            nc.vector.tensor_copy(wu[:], wu0[:])
            wups = pspool.tile([128, 512], F32, tag="ps", name="wups")
            for _ in range(34):
                nc.tensor.matmul(wups[:], wu[:, :128], wu[:], start=True, stop=True)
            wuo = opool.tile([128, 512], F32, tag="o", name="wuo")
            nc.vector.tensor_copy(wuo[:], wups[:])
            nc.sync.dma_start(scratch[:], wuo[:])

            # Gathered tokens: first tiles partition-split across queues so
            # the first real matmul chain starts ASAP.
            xg = []
            for hk in range(HK):
                t = xpool.tile([128, c], F32R, tag=f"xg{hk}", name=f"xg{hk}")
                if hk < 4:
                    nc.gpsimd.dma_start(t[0:64, :], xgT[hk, 0:64])
                    nc.gpsimd.dma_start(t[64:128, :], xgT[hk, 64:128])
                else:
                    nc.gpsimd.dma_start(t[:], xgT[hk])
                xg.append(t)
            hT = [
                hpool.tile([128, c], F32R, tag=f"h{ik}", name=f"h{ik}")
                for ik in range(IK)
            ]

            # phase 1: hT = silu(w1.T @ xgT) * (w3.T @ xgT)
            # 2 * icpt * len(ch) == 8 PSUM accumulation groups per icg;
            # each weight tile is consumed within one hk iteration.
            for icg in range(n_icg):
                ps = {}
                for w in (0, 1):
                    for ic in range(icpt):
                        for ci, (off, sz) in enumerate(ch):
                            ps[w, ic, ci] = pspool.tile(
                                [128, sz], F32, tag="ps", name=f"ps{w}_{ic}_{ci}"
                            )
                if wt_w == 512:
                    # one weight tile per hk: 2KB/partition lines already
                    for hk in range(HK):
                        wt1 = wpool.tile(
                            [128, wt_w], F32R, tag="w1", name=f"w1t{hk}"
                        )
                        nc.sync.dma_start(wt1[:], w1[icg, hk // 2, :, hk % 2])
                        wt3 = wpool.tile(
                            [128, wt_w], F32R, tag="w3", name=f"w3t{hk}"
                        )
                        nc.gpsimd.dma_start(wt3[:], w3[icg, hk // 2, :, hk % 2])
                        for w, wt in ((0, wt1), (1, wt3)):
                            for ic in range(icpt):
                                for ci, (off, sz) in enumerate(ch):
                                    nc.tensor.matmul(
                                        ps[w, ic, ci][:],
                                        wt[:, ic * 128 : (ic + 1) * 128],
                                        xg[hk][:, off : off + sz],
                                        start=(hk == 0),
                                        stop=(hk == HK - 1),
                                    )
                else:
                    # wt_w == 256: pair hk so the DMA block stays at
                    # 2KB/partition contiguous lines
                    for hk0 in range(0, HK, 2):
                        wt1 = wpool.tile(
                            [128, 2, wt_w], F32R, tag="w1", name=f"w1t{hk0}"
                        )
                        nc.sync.dma_start(wt1[:], w1[icg, hk0 // 2])
                        wt3 = wpool.tile(
                            [128, 2, wt_w], F32R, tag="w3", name=f"w3t{hk0}"
                        )
                        nc.gpsimd.dma_start(wt3[:], w3[icg, hk0 // 2])
                        for hh in range(2):
                            hk = hk0 + hh
                            for w, wt in ((0, wt1), (1, wt3)):
                                for ic in range(icpt):
                                    for ci, (off, sz) in enumerate(ch):
                                        nc.tensor.matmul(
                                            ps[w, ic, ci][:],
                                            wt[:, hh, ic * 128 : (ic + 1) * 128],
                                            xg[hk][:, off : off + sz],
                                            start=(hk == 0),
                                            stop=(hk == HK - 1),
                                        )
                for ic in range(icpt):
                    ik = icg * icpt + ic
                    for ci, (off, sz) in enumerate(ch):
                        dst = hT[ik][:, off : off + sz]
                        nc.scalar.activation(dst, ps[0, ic, ci][:], SILU)
                        nc.vector.tensor_mul(dst, dst, ps[1, ic, ci][:])

            # phase 2: y = hT.T @ w2.  Process hg in pairs when PSUM
            # allows, so consecutive matmuls share the same stationary
            # operand.
            hg_grp = 2 if 2 * tt_n <= 8 else 1
            for hg0 in range(0, HG, hg_grp):
                hgs = list(range(hg0, hg0 + hg_grp))
                ps2 = {
                    (tt, hg): pspool.tile(
                        [128, 512], F32, tag="ps", name=f"ps2_{tt}_{hg}"
                    )
                    for tt in range(tt_n)
                    for hg in hgs
                }
                for ik in range(IK):
                    wts = {}
                    for hg in hgs:
                        wt = w2pool.tile(
                            [128, 512], F32R, tag=f"w2_{hg - hg0}",
                            name=f"w2t_{hg}_{ik}",
                        )
                        eng = nc.sync if hg % 2 == 0 else nc.scalar
                        eng.dma_start(wt[:], w2[hg, ik])
                        wts[hg] = wt
                    for tt in range(tt_n):
                        for hg in hgs:
                            nc.tensor.matmul(
                                ps2[tt, hg][:],
                                hT[ik][:, tt * 128 : (tt + 1) * 128],
                                wts[hg][:],
                                start=(ik == 0),
                                stop=(ik == IK - 1),
                            )
                for tt in range(tt_n):
                    for hg in hgs:
                        ot = opool.tile([128, 512], F32, tag="o", name=f"o{tt}_{hg}")
                        nc.vector.tensor_copy(ot[:], ps2[tt, hg][:])
                        nc.sync.dma_start(
                            y[tt * 128 : (tt + 1) * 128, hg * 512 : (hg + 1) * 512],
                            ot[:],
                        )
    nc.compile()
    return nc


def _get_prog(c):
    if c not in _prog_cache:
        _prog_cache[c] = _build(c)
    return _prog_cache[c]


def _retile_weights(w1, w2, w3, wt_w):
    """Host retiling so every device DMA is one contiguous block."""
    n_icg = I // wt_w
    # [E, n_icg, HK//2, 128, 2, wt_w]: (e, icg, hp, p, hh, i) =
    # w[e, (hp*2+hh)*128 + p, icg*wt_w + i]
    w1t = np.ascontiguousarray(
        w1.reshape(E, HK // 2, 2, 128, n_icg, wt_w).transpose(0, 4, 1, 3, 2, 5)
    )
    w3t = np.ascontiguousarray(
        w3.reshape(E, HK // 2, 2, 128, n_icg, wt_w).transpose(0, 4, 1, 3, 2, 5)
    )
    w2t = np.ascontiguousarray(
        w2.reshape(E, IK, 128, HG, 512).transpose(0, 3, 1, 2, 4)
    )
    return w1t, w3t, w2t


def kernel(x, expert_weights, w1, w2, w3, expert_indices):
    x = np.asarray(x, dtype=np.float32)
    expert_weights = np.asarray(expert_weights, dtype=np.float32)
    w1 = np.asarray(w1, dtype=np.float32)
    w2 = np.asarray(w2, dtype=np.float32)
    w3 = np.asarray(w3, dtype=np.float32)
    idx = np.asarray(expert_indices)
    T = x.shape[0]

    # Route: token lists per expert, merging duplicate top-k hits so each
    # token appears at most once per expert (scatter-add safe).
    same = idx[:, 0] == idx[:, 1]
    w_slot0 = np.where(same, expert_weights[:, 0] + expert_weights[:, 1],
                       expert_weights[:, 0])
    toks, wts = [], []
    for e in range(E):
        m0 = idx[:, 0] == e
        m1 = (idx[:, 1] == e) & ~same
        t0 = np.nonzero(m0)[0]
        t1 = np.nonzero(m1)[0]
        toks.append(np.concatenate([t0, t1]))
        wts.append(np.concatenate([w_slot0[m0], expert_weights[m1, 1]]))

    maxcount = max(len(t) for t in toks)
    maxcount = max(maxcount, 1)
    nrounds = -(-maxcount // C_CAP)
    c = -(-(-(-maxcount // nrounds)) // 128) * 128  # ceil to 128
    c = max(c, 128)

    w1t, w3t, w2t = _retile_weights(w1, w2, w3, _wtile(c))
    nc = _get_prog(c)

    out = np.zeros((T, H), dtype=np.float32)
    for r in range(nrounds):
        in_maps = []
        seg_toks = []
        seg_wts = []
        for e in range(E):
            seg = toks[e][r * c : (r + 1) * c]
            sw = wts[e][r * c : (r + 1) * c]
            seg_toks.append(seg)
            seg_wts.append(sw)
            xga = np.zeros((H, c), dtype=np.float32)
            if len(seg):
                xga[:, : len(seg)] = x[seg].T
            in_maps.append(
                {
                    "xgT": np.ascontiguousarray(xga.reshape(HK, 128, c)),
                    "w1": w1t[e],
                    "w3": w3t[e],
                    "w2": w2t[e],
                }
            )
        res = run_bass_kernel_spmd(nc, in_maps, core_ids=list(range(E)))
        for e in range(E):
            seg = seg_toks[e]
            if len(seg) == 0:
                continue
            ye = res.results[e]["y"][: len(seg)]
            out[seg] += ye * seg_wts[e][:, None]
    return out


# revision 35
# speedup vs baseline: 1.1977x; 1.1977x over previous
"""MoE SwiGLU experts (T=2048, H=2048, I=5632, E=8, top-2) on 8 trn2 cores.

Strategy: expert-parallel routed compute. The reference computes all 8
experts densely for every token, but the output only needs each token's
top-2 experts, so we gather tokens per expert on the host (merging the
case where both top-k slots pick the same expert), run one expert per
NeuronCore on its ~T*K/E gathered tokens, and scatter-combine with the
router weights on the host.  4x less device FLOPs than dense.

Per core (expert e), with C = padded token capacity:
  phase 1: hT[i, c] = silu(w1[e].T @ xgT) * (w3[e].T @ xgT)   [I, C]
           - accumulate over 16 H-chunks of 128 in PSUM, f32r matmuls
  phase 2: y[c, h]  = hT.T @ w2[e]                            [C, H]
           - accumulate over 44 I-chunks of 128 in PSUM
All matmuls use float32r (full PE rate at moving-dim >= 256, ~1.4e-4
max rel err vs fp32).  Weights are host-retiled so every DMA is one
contiguous [128, wtile] block (2KB/partition lines when C <= 512).
"""

import numpy as np

import concourse.bacc as bacc
import concourse.mybir as mybir
import concourse.tile as tile
from concourse.bass_utils import run_bass_kernel_spmd

E = 8
H = 2048
I = 5632
HK = H // 128   # 16 contraction chunks for phase 1
IK = I // 128   # 44 contraction chunks for phase 2
HG = H // 512   # 4 output column groups (w2)
C_CAP = 640     # max tokens per expert per round (SBUF budget)

F32 = mybir.dt.float32
F32R = mybir.dt.float32r
SILU = mybir.ActivationFunctionType.Silu

_prog_cache: dict[int, object] = {}


def _chunk_list(c):
    """Split c (multiple of 128) into moving-dim chunks of at most 512
    (PSUM bank), preferring >=256 so f32r matmuls run at full rate."""
    out, off, r = [], 0, c
    while r > 0:
        if r <= 512:
            t = r
        elif r < 768:
            t = r - 256
        else:
            t = 512
        out.append((off, t))
        off += t
        r -= t
    return out


def _wtile(c):
    """Phase-1 weight tile width: 2 weights x (wtile/128) ic-tiles x
    n_chunks accumulation groups must fit in 8 PSUM banks."""
    return 512 if len(_chunk_list(c)) == 1 else 256


def _build(c):
    nc = bacc.Bacc("TRN2", target_bir_lowering=False, debug=False, num_devices=E)
    wt_w = _wtile(c)
    icpt = wt_w // 128          # ic-tiles per phase-1 weight tile
    n_icg = I // wt_w           # phase-1 weight groups
    xgT = nc.dram_tensor("xgT", [HK, 128, c], F32R, kind="ExternalInput")
    w1 = nc.dram_tensor(
        "w1", [n_icg, HK // 2, 128, 2, wt_w], F32R, kind="ExternalInput"
    )
    w3 = nc.dram_tensor(
        "w3", [n_icg, HK // 2, 128, 2, wt_w], F32R, kind="ExternalInput"
    )
    w2 = nc.dram_tensor("w2", [HG, IK, 128, 512], F32R, kind="ExternalInput")
    y = nc.dram_tensor("y", [c, H], F32, kind="ExternalOutput")
    scratch = nc.dram_tensor("scratch", [128, 512], F32, kind="ExternalOutput")

    ch = _chunk_list(c)
    tt_n = c // 128
    wbufs = 6

    with tile.TileContext(nc) as tc:
        with (
            tc.tile_pool(name="xg", bufs=1) as xpool,
            tc.tile_pool(name="h", bufs=1) as hpool,
            tc.tile_pool(name="w", bufs=wbufs) as wpool,
            tc.tile_pool(name="w2p", bufs=8) as w2pool,
            tc.tile_pool(name="ps", bufs=8, space="PSUM") as pspool,
            tc.tile_pool(name="o", bufs=4) as opool,
        ):
            # PE warmup: matmuls on a zeroed tile keep the PE busy (and the
            # HAM clock un-throttled) while the first input DMAs land.
            wu0 = xpool.tile([128, 512], F32, tag="wu0", name="wu0")
            nc.vector.memset(wu0[:], 0.0)
            wu = xpool.tile([128, 512], F32R, tag="wu", name="wu")
            nc.vector.tensor_copy(wu[:], wu0[:])
            wups = pspool.tile([128, 512], F32, tag="ps", name="wups")
            for _ in range(34):
                nc.tensor.matmul(wups[:], wu[:, :128], wu[:], start=True, stop=True)
            wuo = opool.tile([128, 512], F32, tag="o", name="wuo")
            nc.vector.tensor_copy(wuo[:], wups[:])
            nc.sync.dma_start(scratch[:], wuo[:])

            # Gathered tokens: first tiles partition-split across queues so
            # the first real matmul chain starts ASAP.
            xg = []
            for hk in range(HK):
                t = xpool.tile([128, c], F32R, tag=f"xg{hk}", name=f"xg{hk}")
                if hk < 4:
                    nc.sync.dma_start(t[0:64, :], xgT[hk, 0:64])
                    nc.sync.dma_start(t[64:128, :], xgT[hk, 64:128])
                else:
                    nc.sync.dma_start(t[:], xgT[hk])
                xg.append(t)
            hT = [
                hpool.tile([128, c], F32R, tag=f"h{ik}", name=f"h{ik}")
                for ik in range(IK)
            ]

            # phase 1: hT = silu(w1.T @ xgT) * (w3.T @ xgT)
            # 2 * icpt * len(ch) == 8 PSUM accumulation groups per icg;
            # each weight tile is consumed within one hk iteration.
            for icg in range(n_icg):
                ps = {}
                for w in (0, 1):
                    for ic in range(icpt):
                        for ci, (off, sz) in enumerate(ch):
                            ps[w, ic, ci] = pspool.tile(
                                [128, sz], F32, tag="ps", name=f"ps{w}_{ic}_{ci}"
                            )
                if wt_w == 512:
                    # one weight tile per hk: 2KB/partition lines already
                    for hk in range(HK):
                        wt1 = wpool.tile(
                            [128, wt_w], F32R, tag="w1", name=f"w1t{hk}"
                        )
                        nc.sync.dma_start(wt1[:], w1[icg, hk // 2, :, hk % 2])
                        wt3 = wpool.tile(
                            [128, wt_w], F32R, tag="w3", name=f"w3t{hk}"
                        )
                        nc.sync.dma_start(wt3[:], w3[icg, hk // 2, :, hk % 2])
                        for w, wt in ((0, wt1), (1, wt3)):
                            for ic in range(icpt):
                                for ci, (off, sz) in enumerate(ch):
                                    nc.tensor.matmul(
                                        ps[w, ic, ci][:],
                                        wt[:, ic * 128 : (ic + 1) * 128],
                                        xg[hk][:, off : off + sz],
                                        start=(hk == 0),
                                        stop=(hk == HK - 1),
                                    )
                else:
                    # wt_w == 256: pair hk so the DMA block stays at
                    # 2KB/partition contiguous lines
                    for hk0 in range(0, HK, 2):
                        wt1 = wpool.tile(
                            [128, 2, wt_w], F32R, tag="w1", name=f"w1t{hk0}"
                        )
                        nc.sync.dma_start(wt1[:], w1[icg, hk0 // 2])
                        wt3 = wpool.tile(
                            [128, 2, wt_w], F32R, tag="w3", name=f"w3t{hk0}"
                        )
                        nc.sync.dma_start(wt3[:], w3[icg, hk0 // 2])
                        for hh in range(2):
                            hk = hk0 + hh
                            for w, wt in ((0, wt1), (1, wt3)):
                                for ic in range(icpt):
                                    for ci, (off, sz) in enumerate(ch):
                                        nc.tensor.matmul(
                                            ps[w, ic, ci][:],
                                            wt[:, hh, ic * 128 : (ic + 1) * 128],
                                            xg[hk][:, off : off + sz],
                                            start=(hk == 0),
                                            stop=(hk == HK - 1),
                                        )
                for ic in range(icpt):
                    ik = icg * icpt + ic
                    for ci, (off, sz) in enumerate(ch):
                        dst = hT[ik][:, off : off + sz]
                        nc.scalar.activation(dst, ps[0, ic, ci][:], SILU)
                        nc.vector.tensor_mul(dst, dst, ps[1, ic, ci][:])

            # phase 2: y = hT.T @ w2.  Process hg in pairs when PSUM
            # allows, so consecutive matmuls share the same stationary
            # operand.
            hg_grp = 2 if 2 * tt_n <= 8 else 1
            for hg0 in range(0, HG, hg_grp):
                hgs = list(range(hg0, hg0 + hg_grp))
                ps2 = {
                    (tt, hg): pspool.tile(
                        [128, 512], F32, tag="ps", name=f"ps2_{tt}_{hg}"
                    )
                    for tt in range(tt_n)
                    for hg in hgs
                }
                for ik in range(IK):
                    wts = {}
                    for hg in hgs:
                        wt = w2pool.tile(
                            [128, 512], F32R, tag=f"w2_{hg - hg0}",
                            name=f"w2t_{hg}_{ik}",
                        )
                        nc.sync.dma_start(wt[:], w2[hg, ik])
                        wts[hg] = wt
                    for tt in range(tt_n):
                        for hg in hgs:
                            nc.tensor.matmul(
                                ps2[tt, hg][:],
                                hT[ik][:, tt * 128 : (tt + 1) * 128],
                                wts[hg][:],
                                start=(ik == 0),
                                stop=(ik == IK - 1),
                            )
                for tt in range(tt_n):
                    for hg in hgs:
                        ot = opool.tile([128, 512], F32, tag="o", name=f"o{tt}_{hg}")
                        nc.vector.tensor_copy(ot[:], ps2[tt, hg][:])
                        nc.sync.dma_start(
                            y[tt * 128 : (tt + 1) * 128, hg * 512 : (hg + 1) * 512],
                            ot[:],
                        )
    nc.compile()
    return nc


def _get_prog(c):
    if c not in _prog_cache:
        _prog_cache[c] = _build(c)
    return _prog_cache[c]


def _retile_weights(w1, w2, w3, wt_w):
    """Host retiling so every device DMA is one contiguous block."""
    n_icg = I // wt_w
    # [E, n_icg, HK//2, 128, 2, wt_w]: (e, icg, hp, p, hh, i) =
    # w[e, (hp*2+hh)*128 + p, icg*wt_w + i]
    w1t = np.ascontiguousarray(
        w1.reshape(E, HK // 2, 2, 128, n_icg, wt_w).transpose(0, 4, 1, 3, 2, 5)
    )
    w3t = np.ascontiguousarray(
        w3.reshape(E, HK // 2, 2, 128, n_icg, wt_w).transpose(0, 4, 1, 3, 2, 5)
    )
    w2t = np.ascontiguousarray(
        w2.reshape(E, IK, 128, HG, 512).transpose(0, 3, 1, 2, 4)
    )
    return w1t, w3t, w2t


def kernel(x, expert_weights, w1, w2, w3, expert_indices):
    x = np.asarray(x, dtype=np.float32)
    expert_weights = np.asarray(expert_weights, dtype=np.float32)
    w1 = np.asarray(w1, dtype=np.float32)
    w2 = np.asarray(w2, dtype=np.float32)
    w3 = np.asarray(w3, dtype=np.float32)
    idx = np.asarray(expert_indices)
    T = x.shape[0]

    # Route: token lists per expert, merging duplicate top-k hits so each
    # token appears at most once per expert (scatter-add safe).
    same = idx[:, 0] == idx[:, 1]
    w_slot0 = np.where(same, expert_weights[:, 0] + expert_weights[:, 1],
                       expert_weights[:, 0])
    toks, wts = [], []
    for e in range(E):
        m0 = idx[:, 0] == e
        m1 = (idx[:, 1] == e) & ~same
        t0 = np.nonzero(m0)[0]
        t1 = np.nonzero(m1)[0]
        toks.append(np.concatenate([t0, t1]))
        wts.append(np.concatenate([w_slot0[m0], expert_weights[m1, 1]]))

    maxcount = max(len(t) for t in toks)
    maxcount = max(maxcount, 1)
    nrounds = -(-maxcount // C_CAP)
    c = -(-(-(-maxcount // nrounds)) // 128) * 128  # ceil to 128
    c = max(c, 128)

    w1t, w3t, w2t = _retile_weights(w1, w2, w3, _wtile(c))
    nc = _get_prog(c)

    out = np.zeros((T, H), dtype=np.float32)
    for r in range(nrounds):
        in_maps = []
        seg_toks = []
        seg_wts = []
        for e in range(E):
            seg = toks[e][r * c : (r + 1) * c]
            sw = wts[e][r * c : (r + 1) * c]
            seg_toks.append(seg)
            seg_wts.append(sw)
            xga = np.zeros((H, c), dtype=np.float32)
            if len(seg):
                xga[:, : len(seg)] = x[seg].T
            in_maps.append(
                {
                    "xgT": np.ascontiguousarray(xga.reshape(HK, 128, c)),
                    "w1": w1t[e],
                    "w3": w3t[e],
                    "w2": w2t[e],
                }
            )
        res = run_bass_kernel_spmd(nc, in_maps, core_ids=list(range(E)))
        for e in range(E):
            seg = seg_toks[e]
            if len(seg) == 0:
                continue
            ye = res.results[e]["y"][: len(seg)]
            out[seg] += ye * seg_wts[e][:, None]
    return out
